# revision 1
# baseline (speedup 1.0000x reference)
"""Trainium2 Bass kernel for a 2-layer aspect-gated GCN (AspectOrientedDepGCN).

Strategy (8 NeuronCores, SPMD):
  - Nodes sharded across cores (6250/core, padded to 6272 = 49*128).
  - Every core holds a full bf16 replica of the layer input x for gathers.
  - Edges partitioned by dst partition, sorted by dst, chunked into groups of
    128; scatter-add is computed as one-hot-matrix matmuls accumulated in PSUM.
  - Per layer: indirect-DMA gather -> scatter matmuls -> PE transpose to
    aggT [D, nodes] -> weight-stationary matmuls (x_gcn^T, gate^T) ->
    gated combine in transposed layout -> PE transpose back -> LayerNorm
    (bn_stats) -> DRAM.
  - One bf16 AllGather of x1 between the layers (7 chunks, pipelined).
All matmuls run in bf16 with fp32 PSUM accumulation; LayerNorm in fp32.
"""
import sys

sys.path.insert(0, "/opt/trn_rl_repo")

import numpy as np
import ml_dtypes

D = 768
KT = 6          # D / 128
EPS = 1e-5
L = 2


class Cfg:
    def __init__(self, n_nodes, n_cores, ag_ch, gather_r=4):
        self.n_nodes = n_nodes
        self.n_cores = n_cores
        self.p_local = n_nodes // n_cores          # real nodes per core
        assert self.p_local * n_cores == n_nodes
        self.nblk = (self.p_local + 127) // 128    # 128-node dst blocks
        self.p_pad = self.nblk * 128               # padded nodes per core
        self.n_full = self.p_pad * n_cores
        self.ag_ch = ag_ch                         # AllGather chunks
        assert self.p_pad % ag_ch == 0
        self.ag_rows = self.p_pad // ag_ch
        assert self.ag_rows % 128 == 0
        self.gather_r = gather_r
        # node columns for the transposed main matmuls: 512-wide + remainder
        cols = []
        o = 0
        while o < self.p_pad:
            w = min(512, self.p_pad - o)
            cols.append((o, w))
            o += w
        assert all(w % 128 == 0 for _, w in cols)
        self.cols = cols


FULL = Cfg(50000, 8, ag_ch=7)


# ---------------------------------------------------------------- host prep

def _pad_global(cfg, g):
    """global node id -> row in the padded replicated layout [n_full, D]."""
    return (g // cfg.p_local) * cfg.p_pad + g % cfg.p_local


def _ag_reorder(cfg, g):
    """global node id -> row in the AllGather output layout
    [ag_ch][n_cores][ag_rows][D]."""
    r = g // cfg.p_local
    l = g % cfg.p_local
    ch = l // cfg.ag_rows
    return ch * (cfg.n_cores * cfg.ag_rows) + r * cfg.ag_rows + (l % cfg.ag_rows)


def prep(cfg, inputs):
    """Split edges by dst partition, build chunk schedule + per-core packed
    index / one-hot arrays and weight layouts."""
    edge = np.asarray(inputs["edge_index"])
    src_g = edge[0].astype(np.int64)
    dst_g = edge[1].astype(np.int64)
    nc_ = cfg.n_cores

    per_core = []
    counts = np.zeros((nc_, cfg.nblk), np.int64)
    for c in range(nc_):
        m = (dst_g // cfg.p_local) == c
        s = src_g[m]
        d = dst_g[m] - c * cfg.p_local
        order = np.argsort(d, kind="stable")
        s, d = s[order], d[order]
        per_core.append((s, d))
        counts[c] = np.bincount(d // 128, minlength=cfg.nblk)

    # shared chunk schedule: per block, chunks = max over cores
    cb = np.maximum(1, -(-counts.max(axis=0) // 128))   # ceil, min 1
    offs = np.concatenate([[0], np.cumsum(cb)])         # chunk offset per block
    c_total = int(offs[-1])
    ngroups = -(-c_total // cfg.gather_r)
    c_pad = ngroups * cfg.gather_r

    src_packs, s_hosts = [], []
    for c in range(nc_):
        s, d = per_core[c]
        src_slots = np.zeros(c_pad * 128, np.int64)
        scol = np.full(c_pad * 128, -1, np.int64)
        blk = d // 128
        starts = np.concatenate([[0], np.cumsum(np.bincount(blk, minlength=cfg.nblk))])
        for b in range(cfg.nblk):
            e0, e1 = starts[b], starts[b + 1]
            base = offs[b] * 128
            n = e1 - e0
            src_slots[base:base + n] = s[e0:e1]
            scol[base:base + n] = d[e0:e1] - b * 128
        # S one-hot: [128 partitions (edge slot), c_pad*128]
        sh = np.zeros((128, c_pad * 128), ml_dtypes.bfloat16)
        cc = np.arange(c_pad * 128)
        valid = scol >= 0
        p_ = cc[valid] % 128
        ch_ = cc[valid] // 128
        sh[p_, ch_ * 128 + scol[valid]] = 1.0
        # src packed [128, c_pad]
        sp = src_slots.reshape(c_pad, 128).T.copy()
        src_packs.append(sp)
        s_hosts.append(sh)

    # weights
    def pack_w(w):  # [D, D] -> [128, KT, D] bf16 (partition-major per k tile)
        return np.ascontiguousarray(
            w.reshape(KT, 128, D).transpose(1, 0, 2)).astype(ml_dtypes.bfloat16)

    def pack_v(v, dt=np.float32):  # [D] -> [128, KT]
        return np.ascontiguousarray(v.reshape(KT, 128).T).astype(dt)

    gcn_w = np.asarray(inputs["gcn_w"], np.float32)
    gate_w = np.asarray(inputs["gate_w"], np.float32)
    x0 = np.asarray(inputs["token_embeddings"], np.float32)

    # padded bf16 replica of x0 and per-core own slices
    x0_rep = np.zeros((cfg.n_full, D), ml_dtypes.bfloat16)
    for c in range(nc_):
        x0_rep[c * cfg.p_pad:c * cfg.p_pad + cfg.p_local] = \
            x0[c * cfg.p_local:(c + 1) * cfg.p_local]

    shared = {
        "w0": pack_w(gcn_w[0]),
        "w1": pack_w(gcn_w[1]),
        "gwt": pack_w(gate_w[:D]),
        "gwb": pack_w(gate_w[D:]),
        "aspect": pack_v(np.asarray(inputs["aspect_embedding"]), ml_dtypes.bfloat16),
        "b0": pack_v(np.asarray(inputs["gcn_b"])[0]),
        "b1": pack_v(np.asarray(inputs["gcn_b"])[1]),
        "gb": pack_v(np.asarray(inputs["gate_b"])),
        "gam": np.broadcast_to(
            np.asarray(inputs["ln_gamma"]).astype(ml_dtypes.bfloat16)[None],
            (128, L, D)).copy(),
        "bet": np.broadcast_to(
            np.asarray(inputs["ln_beta"]).astype(ml_dtypes.bfloat16)[None],
            (128, L, D)).copy(),
    }
    in_maps = []
    for c in range(nc_):
        m = dict(shared)
        m["x0own"] = np.ascontiguousarray(
            x0_rep[c * cfg.p_pad:(c + 1) * cfg.p_pad])
        src1 = _pad_global(cfg, src_packs[c]).astype(np.int32)
        # L1 messages pre-arranged, partition-major: contiguous per-partition
        m["x0arr"] = np.ascontiguousarray(x0_rep[src1])   # [128, c_pad, D]
        m["src2"] = _ag_reorder(cfg, src_packs[c]).astype(np.int32)
        m["smat"] = s_hosts[c]
        in_maps.append(m)

    sched = {"cb": cb.astype(int).tolist(), "offs": offs.astype(int).tolist(),
             "c_total": c_total, "c_pad": c_pad, "ngroups": ngroups}
    return in_maps, sched


# ---------------------------------------------------------------- builder

def build(cfg, sched, dbg=False, skip_ag=False, plain_gather=False,
          skip_scatter=False, dummy_in=False):
    import concourse.bass as bass
    import concourse.tile as tile
    from concourse import bacc, mybir
    from concourse.masks import make_identity

    f32 = mybir.dt.float32
    bf16 = mybir.dt.bfloat16
    i32 = mybir.dt.int32
    AF = mybir.ActivationFunctionType
    AL = mybir.AluOpType

    cb, offs = sched["cb"], sched["offs"]
    c_pad, ngroups = sched["c_pad"], sched["ngroups"]
    R = cfg.gather_r

    nc = bacc.Bacc("TRN2", target_bir_lowering=False, debug=False,
                   num_devices=cfg.n_cores)

    x0arr_ext = nc.dram_tensor("x0arr", [128, c_pad, D], bf16,
                               kind="ExternalInput")
    if dummy_in:
        nc.dram_tensor("dummy", [50176, D], bf16, kind="ExternalInput")
    x0own_ext = nc.dram_tensor("x0own", [cfg.p_pad, D], bf16, kind="ExternalInput")
    src_ext = [None,
               nc.dram_tensor("src2", [128, c_pad], i32, kind="ExternalInput")]
    s_ext = nc.dram_tensor("smat", [128, c_pad * 128], bf16, kind="ExternalInput")
    w_ext = [nc.dram_tensor("w0", [128, KT, D], bf16, kind="ExternalInput"),
             nc.dram_tensor("w1", [128, KT, D], bf16, kind="ExternalInput")]
    gwt_ext = nc.dram_tensor("gwt", [128, KT, D], bf16, kind="ExternalInput")
    gwb_ext = nc.dram_tensor("gwb", [128, KT, D], bf16, kind="ExternalInput")
    asp_ext = nc.dram_tensor("aspect", [128, KT], bf16, kind="ExternalInput")
    b_ext = [nc.dram_tensor("b0", [128, KT], f32, kind="ExternalInput"),
             nc.dram_tensor("b1", [128, KT], f32, kind="ExternalInput")]
    gb_ext = nc.dram_tensor("gb", [128, KT], f32, kind="ExternalInput")
    gam_ext = nc.dram_tensor("gam", [128, L, D], bf16, kind="ExternalInput")
    bet_ext = nc.dram_tensor("bet", [128, L, D], bf16, kind="ExternalInput")
    out_ext = nc.dram_tensor("out", [cfg.p_pad, D], f32, kind="ExternalOutput")

    if dbg:
        dbg_aggT = nc.dram_tensor("dbg_aggT", [128, KT, cfg.p_pad], bf16,
                                  kind="ExternalOutput")
        dbg_x1 = nc.dram_tensor("dbg_x1", [cfg.p_pad, D], bf16,
                                kind="ExternalOutput")
        dbg_msgs = nc.dram_tensor("dbg_msgs", [128, 4, D], bf16,
                                  kind="ExternalOutput")
    x1_own = nc.dram_tensor("x1_own", [cfg.p_pad, D], bf16)
    x1_full = nc.dram_tensor("x1_full", [cfg.n_full, D], bf16,
                             addr_space="Shared")

    with tile.TileContext(nc) as tc:
        with tc.tile_pool(name="single", bufs=1) as single, \
             tc.tile_pool(name="aggT", bufs=1) as aggT_p, \
             tc.tile_pool(name="wrot", bufs=1) as wrot, \
             tc.tile_pool(name="lnc", bufs=1) as lnc, \
             tc.tile_pool(name="msgs", bufs=4) as msgs_p, \
             tc.tile_pool(name="sblk", bufs=4) as s_p, \
             tc.tile_pool(name="aggbf", bufs=3) as aggbf_p, \
             tc.tile_pool(name="colt", bufs=2) as col_p, \
             tc.tile_pool(name="nat", bufs=4) as nat_p, \
             tc.tile_pool(name="lns", bufs=4) as lns_p, \
             tc.tile_pool(name="psA", bufs=1, space="PSUM") as psA, \
             tc.tile_pool(name="psT", bufs=2, space="PSUM") as psT, \
             tc.tile_pool(name="psM", bufs=3, space="PSUM") as psM:

            ident = single.tile([128, 128], bf16, tag="ident")
            make_identity(nc, ident[:])
            eps_t = single.tile([128, 1], f32, tag="eps")
            nc.vector.memset(eps_t[:], EPS)

            gwt_t = single.tile([128, KT, D], bf16, tag="gwt")
            nc.sync.dma_start(out=gwt_t[:], in_=gwt_ext[:, :, :])
            asp_t = single.tile([128, KT], bf16, tag="asp")
            nc.sync.dma_start(out=asp_t[:], in_=asp_ext[:, :])
            gb_t = single.tile([128, KT], f32, tag="gb")
            nc.sync.dma_start(out=gb_t[:], in_=gb_ext[:, :])
            b_t = single.tile([128, 2, KT], f32, tag="bias")
            nc.sync.dma_start(out=b_t[:, 0, :], in_=b_ext[0][:, :])
            nc.sync.dma_start(out=b_t[:, 1, :], in_=b_ext[1][:, :])
            src_t = single.tile([128, 2, c_pad], i32, tag="src")
            nc.sync.dma_start(out=src_t[:, 1, :], in_=src_ext[1][:, :])
            geff_t = single.tile([128, KT], f32, tag="geff")

            # gate bias fold: geff = aspect @ gate_w[D:] + gate_b
            gwb_t = wrot.tile([128, KT, D], bf16, tag="wl")
            nc.sync.dma_start(out=gwb_t[:], in_=gwb_ext[:, :, :])
            for m in range(KT):
                ps = psM.tile([128, 512], f32, tag="mps")
                for k in range(KT):
                    nc.tensor.matmul(out=ps[:, 0:1],
                                     lhsT=gwb_t[:, k, m * 128:(m + 1) * 128],
                                     rhs=asp_t[:, k:k + 1],
                                     start=(k == 0), stop=(k == KT - 1))
                nc.scalar.activation(out=geff_t[:, m:m + 1], in_=ps[:, 0:1],
                                     func=AF.Identity, bias=gb_t[:, m:m + 1])

            for l in range(L):
                x_src = x0arr_ext if l == 0 else x1_full
                xold_src = x0own_ext if l == 0 else x1_own

                w_t = wrot.tile([128, KT, D], bf16, tag="wl")
                nc.sync.dma_start(out=w_t[:], in_=w_ext[l][:, :, :])
                gam_t = lnc.tile([128, D], bf16, tag="gam")
                nc.sync.dma_start(out=gam_t[:], in_=gam_ext[:, l, :])
                bet_t = lnc.tile([128, D], bf16, tag="bet")
                nc.sync.dma_start(out=bet_t[:], in_=bet_ext[:, l, :])

                # ---- phase A: gather + scatter + transpose -> aggT
                aggT = aggT_p.tile([128, KT, cfg.p_pad], bf16, tag="aggT")
                mtiles = {}
                for g in range(ngroups):
                    mt = msgs_p.tile([128, R, D], bf16, tag="msgs")
                    if l == 0:
                        nc.sync.dma_start(
                            out=mt[:],
                            in_=x0arr_ext[:, g * R:(g + 1) * R, :])
                    elif plain_gather:
                        base = (g * R * 128) % (cfg.n_full - R * 128)
                        nc.gpsimd.dma_start(
                            out=mt[:],
                            in_=x_src[base:base + R * 128, :].rearrange(
                                "(r p) d -> p r d", p=128))
                    else:
                        for r in range(R):
                            nc.gpsimd.indirect_dma_start(
                                out=mt[:, r, :], out_offset=None,
                                in_=x_src[:, :],
                                in_offset=bass.IndirectOffsetOnAxis(
                                    ap=src_t[:, l, g * R + r:g * R + r + 1], axis=0))
                    mtiles[g] = mt

                if skip_scatter:
                    nc.vector.memset(aggT[:, 0, 0:2], 0.0)
                for b in range(cfg.nblk if not skip_scatter else 0):
                    cbb = cb[b]
                    s_t = s_p.tile([128, max(cb) * 128], bf16, tag="sblk")
                    nc.sync.dma_start(
                        out=s_t[:, :cbb * 128],
                        in_=s_ext[:, offs[b] * 128:(offs[b] + cbb) * 128])
                    agg_ps = psA.tile([128, D], f32, tag="aps")
                    for j in range(cbb):
                        c = offs[b] + j
                        mt = mtiles[c // R]
                        jj = c % R
                        nc.tensor.matmul(out=agg_ps[:, 0:512],
                                         lhsT=s_t[:, j * 128:(j + 1) * 128],
                                         rhs=mt[:, jj, 0:512],
                                         start=(j == 0), stop=(j == cbb - 1))
                        nc.tensor.matmul(out=agg_ps[:, 512:D],
                                         lhsT=s_t[:, j * 128:(j + 1) * 128],
                                         rhs=mt[:, jj, 512:D],
                                         start=(j == 0), stop=(j == cbb - 1))
                    agg_bf = aggbf_p.tile([128, D], bf16, tag="aggbf")
                    nc.scalar.copy(out=agg_bf[:], in_=agg_ps[:])
                    for k in range(0, KT, 2):
                        tp = psT.tile([128, 2, 128], bf16, tag="tps")
                        nc.tensor.transpose(out=tp[:, 0, :],
                                            in_=agg_bf[:, k * 128:(k + 1) * 128],
                                            identity=ident[:])
                        nc.tensor.transpose(out=tp[:, 1, :],
                                            in_=agg_bf[:, (k + 1) * 128:(k + 2) * 128],
                                            identity=ident[:])
                        nc.vector.tensor_copy(
                            out=aggT[:, k:k + 2, b * 128:(b + 1) * 128], in_=tp[:])

                if dbg and l == 0:
                    nc.sync.dma_start(out=dbg_aggT[:, :, :], in_=aggT[:])
                    nc.gpsimd.dma_start(out=dbg_msgs[:, :, :], in_=mtiles[0][:])
                # ---- phase B: matmuls + gate + combine + LN per node column
                ag_done = 0
                for (o, w) in cfg.cols:
                    xoldT = col_p.tile([128, KT, 512], bf16, tag="xoldT")
                    for k in range(KT):
                        nc.sync.dma_start_transpose(
                            out=xoldT[:, k, :w],
                            in_=xold_src[o:o + w, k * 128:(k + 1) * 128])
                    xgT = col_p.tile([128, KT, 512], bf16, tag="xgT")
                    for m in range(KT):
                        ps = psM.tile([128, 512], f32, tag="mps")
                        for k in range(KT):
                            nc.tensor.matmul(out=ps[:, :w],
                                             lhsT=w_t[:, k, m * 128:(m + 1) * 128],
                                             rhs=aggT[:, k, o:o + w],
                                             start=(k == 0), stop=(k == KT - 1))
                        nc.scalar.activation(out=xgT[:, m, :w], in_=ps[:, :w],
                                             func=AF.Relu, bias=b_t[:, l, m:m + 1])
                    gT = col_p.tile([128, KT, 512], bf16, tag="gT")
                    for m in range(KT):
                        ps = psM.tile([128, 512], f32, tag="mps")
                        for k in range(KT):
                            nc.tensor.matmul(out=ps[:, :w],
                                             lhsT=gwt_t[:, k, m * 128:(m + 1) * 128],
                                             rhs=xgT[:, k, :w],
                                             start=(k == 0), stop=(k == KT - 1))
                        nc.scalar.activation(out=gT[:, m, :w], in_=ps[:, :w],
                                             func=AF.Sigmoid, bias=geff_t[:, m:m + 1])
                    # combine in place into xgT: xn = g*(xg - xo) + xo
                    nc.vector.tensor_sub(xgT[:, :, :w], xgT[:, :, :w],
                                         xoldT[:, :, :w])
                    nc.vector.tensor_mul(xgT[:, :, :w], gT[:, :, :w],
                                         xgT[:, :, :w])
                    nc.vector.tensor_add(xgT[:, :, :w], xgT[:, :, :w],
                                         xoldT[:, :, :w])
                    # transpose back + LN + store
                    for sub in range(w // 128):
                        nat = nat_p.tile([128, D], bf16, tag="nat")
                        for k in range(0, KT, 2):
                            tp = psT.tile([128, 2, 128], bf16, tag="tps")
                            nc.tensor.transpose(
                                out=tp[:, 0, :],
                                in_=xgT[:, k, sub * 128:(sub + 1) * 128],
                                identity=ident[:])
                            nc.tensor.transpose(
                                out=tp[:, 1, :],
                                in_=xgT[:, k + 1, sub * 128:(sub + 1) * 128],
                                identity=ident[:])
                            nc.vector.tensor_copy(
                                out=nat[:, k * 128:(k + 2) * 128], in_=tp[:])
                        stats = lns_p.tile([128, 3, 6], f32, tag="stats")
                        for gi in range(3):
                            nc.vector.bn_stats(
                                out=stats[:, gi, :],
                                in_=nat[:, 256 * gi:256 * (gi + 1)])
                        mv = lns_p.tile([128, 2], f32, tag="mv")
                        nc.vector.bn_aggr(out=mv[:], in_=stats[:])
                        rstd = lns_p.tile([128, 1], f32, tag="rstd")
                        nc.scalar.activation(out=rstd[:], in_=mv[:, 1:2],
                                             func=AF.Sqrt, bias=eps_t[:])
                        nc.vector.reciprocal(out=rstd[:], in_=rstd[:])
                        nc.vector.tensor_scalar(out=nat[:], in0=nat[:],
                                                scalar1=mv[:, 0:1],
                                                scalar2=rstd[:],
                                                op0=AL.subtract, op1=AL.mult)
                        nc.vector.tensor_mul(nat[:], nat[:], gam_t[:])
                        r0 = o + sub * 128
                        if l == 0:
                            xnb = nat_p.tile([128, D], bf16, tag="natbf")
                            nc.vector.tensor_add(xnb[:], nat[:], bet_t[:])
                            nc.sync.dma_start(out=x1_own[r0:r0 + 128, :],
                                              in_=xnb[:])
                        else:
                            natf = nat_p.tile([128, D], f32, tag="natf")
                            nc.vector.tensor_add(natf[:], nat[:], bet_t[:])
                            nc.sync.dma_start(out=out_ext[r0:r0 + 128, :],
                                              in_=natf[:])
                    # issue any AllGather chunks whose rows are complete
                    if l == 0:
                        if dbg and (o, w) == cfg.cols[-1]:
                            pass
                        done_rows = o + w
                        while (not skip_ag and ag_done < cfg.ag_ch
                               and (ag_done + 1) * cfg.ag_rows <= done_rows):
                            g = ag_done
                            nc.gpsimd.collective_compute(
                                "AllGather",
                                mybir.AluOpType.bypass,
                                replica_groups=[list(range(cfg.n_cores))],
                                ins=[x1_own[g * cfg.ag_rows:(g + 1) * cfg.ag_rows, :]],
                                outs=[x1_full[g * cfg.n_cores * cfg.ag_rows:
                                              (g + 1) * cfg.n_cores * cfg.ag_rows, :]],
                            )
                            ag_done += 1

            if dbg:
                nc.sync.dma_start(out=dbg_x1[:, :], in_=x1_own[:, :])
    nc.compile()
    return nc


# ---------------------------------------------------------------- entry

def _run(inputs, cfg=FULL, trace=False):
    from concourse.bass_utils import run_bass_kernel_spmd
    in_maps, sched = prep(cfg, inputs)
    nc = build(cfg, sched)
    res = run_bass_kernel_spmd(nc, in_maps, core_ids=list(range(cfg.n_cores)),
                               trace=trace)
    outs = [res.results[c]["out"][:cfg.p_local] for c in range(cfg.n_cores)]
    full = np.concatenate(outs, axis=0).astype(np.float32)
    return full, res


def kernel(**inputs):
    out, _ = _run(inputs, FULL, trace=False)
    return out



# revision 2
# speedup vs baseline: 1.0850x; 1.0850x over previous
"""Trainium2 Bass kernel for a 2-layer aspect-gated GCN (AspectOrientedDepGCN).

Strategy (8 NeuronCores, SPMD):
  - Nodes sharded across cores (6250/core, padded to 6272 = 49*128).
  - Edges partitioned by dst partition, sorted by dst, chunked into groups of
    128; scatter-add is computed as one-hot-matrix matmuls accumulated in PSUM.
  - Per layer: gather -> scatter matmuls -> PE transpose to aggT [D, nodes] ->
    weight-stationary matmuls (x_gcn^T, gate^T) -> gated combine in transposed
    layout -> PE transpose back -> LayerNorm (bn_stats) -> DRAM.
  - Layer-2 messages are routed with a single AllToAll of only the rows each
    core actually needs (deduped per (owner, receiver) pair), instead of
    AllGathering the full x1: sender indirect-gathers its x1 rows into
    per-destination send blocks, one 8-way AllToAll moves them, receivers
    indirect-gather messages from the received table.
All matmuls run in bf16 with fp32 PSUM accumulation; LayerNorm in fp32.
"""
import sys

sys.path.insert(0, "/opt/trn_rl_repo")

import numpy as np
import ml_dtypes

D = 768
KT = 6          # D / 128
EPS = 1e-5
L = 2


class Cfg:
    def __init__(self, n_nodes, n_cores, gather_r=4):
        self.n_nodes = n_nodes
        self.n_cores = n_cores
        self.p_local = n_nodes // n_cores          # real nodes per core
        assert self.p_local * n_cores == n_nodes
        self.nblk = (self.p_local + 127) // 128    # 128-node dst blocks
        self.p_pad = self.nblk * 128               # padded nodes per core
        self.n_full = self.p_pad * n_cores
        self.gather_r = gather_r
        # node columns for the transposed main matmuls: 512-wide + remainder
        cols = []
        o = 0
        while o < self.p_pad:
            w = min(512, self.p_pad - o)
            cols.append((o, w))
            o += w
        assert all(w % 128 == 0 for _, w in cols)
        self.cols = cols


FULL = Cfg(50000, 8)


# ---------------------------------------------------------------- host prep

def prep(cfg, inputs):
    """Split edges by dst partition, build chunk schedule + per-core packed
    index / one-hot arrays, AllToAll routing tables, and weight layouts."""
    edge = np.asarray(inputs["edge_index"])
    src_g = edge[0].astype(np.int64)
    dst_g = edge[1].astype(np.int64)
    nc_ = cfg.n_cores

    per_core = []
    counts = np.zeros((nc_, cfg.nblk), np.int64)
    for c in range(nc_):
        m = (dst_g // cfg.p_local) == c
        s = src_g[m]
        d = dst_g[m] - c * cfg.p_local
        order = np.argsort(d, kind="stable")
        s, d = s[order], d[order]
        per_core.append((s, d))
        counts[c] = np.bincount(d // 128, minlength=cfg.nblk)

    # shared chunk schedule: per block, chunks = max over cores
    cb = np.maximum(1, -(-counts.max(axis=0) // 128))   # ceil, min 1
    offs = np.concatenate([[0], np.cumsum(cb)])         # chunk offset per block
    c_total = int(offs[-1])
    ngroups = -(-c_total // cfg.gather_r)
    c_pad = ngroups * cfg.gather_r

    # ---- pack per-core edge slots (slot -> global src id, one-hot S)
    src_packs, s_hosts = [], []
    for c in range(nc_):
        s, d = per_core[c]
        src_slots = np.zeros(c_pad * 128, np.int64)
        scol = np.full(c_pad * 128, -1, np.int64)
        blk = d // 128
        starts = np.concatenate([[0], np.cumsum(np.bincount(blk, minlength=cfg.nblk))])
        for b in range(cfg.nblk):
            e0, e1 = starts[b], starts[b + 1]
            base = offs[b] * 128
            n = e1 - e0
            src_slots[base:base + n] = s[e0:e1]
            scol[base:base + n] = d[e0:e1] - b * 128
        # S one-hot: [128 partitions (edge slot), c_pad*128]
        sh = np.zeros((128, c_pad * 128), ml_dtypes.bfloat16)
        cc = np.arange(c_pad * 128)
        valid = scol >= 0
        p_ = cc[valid] % 128
        ch_ = cc[valid] // 128
        sh[p_, ch_ * 128 + scol[valid]] = 1.0
        # src packed [128, c_pad]; invalid slots hold src 0 (harmless row)
        sp = src_slots.reshape(c_pad, 128).T.copy()
        src_packs.append(sp)
        s_hosts.append(sh)
        mask = valid.reshape(c_pad, 128).T.copy()
        if c == 0:
            masks = [mask]
        else:
            masks.append(mask)

    # ---- AllToAll routing for layer 2
    # pairlist[o][c]: sorted unique global src ids owned by o, needed by c
    pairlist = [[None] * nc_ for _ in range(nc_)]
    bp_need = 0
    for c in range(nc_):
        s = per_core[c][0]
        for o in range(nc_):
            u = np.unique(s[(s // cfg.p_local) == o])
            pairlist[o][c] = u
            bp_need = max(bp_need, len(u))
    bp = -(-bp_need // 128) * 128                 # pair block rows (padded)
    nst = nc_ * bp // 128                          # send tiles of 128 rows

    send_idx, recv_idx = [], []
    for me in range(nc_):
        # sender: sendbuf row (dest*bp + k) <- x1_own row
        sidx = np.zeros(nc_ * bp, np.int64)
        for dest in range(nc_):
            u = pairlist[me][dest]
            sidx[dest * bp:dest * bp + len(u)] = u % cfg.p_local
        send_idx.append(np.ascontiguousarray(
            sidx.reshape(nst, 128).T).astype(np.int32))
        # receiver: slot (global src id) -> recvbuf row (owner*bp + rank)
        lut = {}
        for o in range(nc_):
            for k, sgid in enumerate(pairlist[o][me]):
                lut[int(sgid)] = o * bp + k
        sp = src_packs[me]
        ridx = np.zeros_like(sp)
        m = masks[me]
        flat = sp.ravel()
        rflat = np.array([lut.get(int(g), 0) for g in flat], np.int64)
        ridx = rflat.reshape(sp.shape)
        ridx[~m] = 0
        recv_idx.append(ridx.astype(np.int32))

    # ---- weights
    def pack_w(w):  # [D, D] -> [128, KT, D] bf16 (partition-major per k tile)
        return np.ascontiguousarray(
            w.reshape(KT, 128, D).transpose(1, 0, 2)).astype(ml_dtypes.bfloat16)

    def pack_v(v, dt=np.float32):  # [D] -> [128, KT]
        return np.ascontiguousarray(v.reshape(KT, 128).T).astype(dt)

    gcn_w = np.asarray(inputs["gcn_w"], np.float32)
    gate_w = np.asarray(inputs["gate_w"], np.float32)
    x0 = np.asarray(inputs["token_embeddings"], np.float32)

    # padded bf16 replica of x0 and per-core own slices
    x0_rep = np.zeros((cfg.n_full, D), ml_dtypes.bfloat16)
    for c in range(nc_):
        x0_rep[c * cfg.p_pad:c * cfg.p_pad + cfg.p_local] = \
            x0[c * cfg.p_local:(c + 1) * cfg.p_local]

    def _pad_global(g):
        return (g // cfg.p_local) * cfg.p_pad + g % cfg.p_local

    shared = {
        "w0": pack_w(gcn_w[0]),
        "w1": pack_w(gcn_w[1]),
        "gwt": pack_w(gate_w[:D]),
        "gwb": pack_w(gate_w[D:]),
        "aspect": pack_v(np.asarray(inputs["aspect_embedding"]), ml_dtypes.bfloat16),
        "b0": pack_v(np.asarray(inputs["gcn_b"])[0]),
        "b1": pack_v(np.asarray(inputs["gcn_b"])[1]),
        "gb": pack_v(np.asarray(inputs["gate_b"])),
        "gam": np.broadcast_to(
            np.asarray(inputs["ln_gamma"]).astype(ml_dtypes.bfloat16)[None],
            (128, L, D)).copy(),
        "bet": np.broadcast_to(
            np.asarray(inputs["ln_beta"]).astype(ml_dtypes.bfloat16)[None],
            (128, L, D)).copy(),
    }
    in_maps = []
    for c in range(nc_):
        m = dict(shared)
        m["x0own"] = np.ascontiguousarray(
            x0_rep[c * cfg.p_pad:(c + 1) * cfg.p_pad])
        src1 = _pad_global(src_packs[c]).astype(np.int32)
        # L1 messages pre-arranged, partition-major: contiguous per-partition
        m["x0arr"] = np.ascontiguousarray(x0_rep[src1])   # [128, c_pad, D]
        m["src2"] = recv_idx[c]
        m["sendidx"] = send_idx[c]
        m["smat"] = s_hosts[c]
        in_maps.append(m)

    sched = {"cb": cb.astype(int).tolist(), "offs": offs.astype(int).tolist(),
             "c_total": c_total, "c_pad": c_pad, "ngroups": ngroups,
             "bp": bp, "nst": nst}
    return in_maps, sched


# ---------------------------------------------------------------- builder

def build(cfg, sched, dbg=False):
    import concourse.bass as bass
    import concourse.tile as tile
    from concourse import bacc, mybir
    from concourse.masks import make_identity

    f32 = mybir.dt.float32
    bf16 = mybir.dt.bfloat16
    i32 = mybir.dt.int32
    AF = mybir.ActivationFunctionType
    AL = mybir.AluOpType

    cb, offs = sched["cb"], sched["offs"]
    c_pad, ngroups = sched["c_pad"], sched["ngroups"]
    bp, nst = sched["bp"], sched["nst"]
    R = cfg.gather_r

    nc = bacc.Bacc("TRN2", target_bir_lowering=False, debug=False,
                   num_devices=cfg.n_cores)

    x0arr_ext = nc.dram_tensor("x0arr", [128, c_pad, D], bf16,
                               kind="ExternalInput")
    x0own_ext = nc.dram_tensor("x0own", [cfg.p_pad, D], bf16, kind="ExternalInput")
    src_ext = [None,
               nc.dram_tensor("src2", [128, c_pad], i32, kind="ExternalInput")]
    sendidx_ext = nc.dram_tensor("sendidx", [128, nst], i32, kind="ExternalInput")
    s_ext = nc.dram_tensor("smat", [128, c_pad * 128], bf16, kind="ExternalInput")
    w_ext = [nc.dram_tensor("w0", [128, KT, D], bf16, kind="ExternalInput"),
             nc.dram_tensor("w1", [128, KT, D], bf16, kind="ExternalInput")]
    gwt_ext = nc.dram_tensor("gwt", [128, KT, D], bf16, kind="ExternalInput")
    gwb_ext = nc.dram_tensor("gwb", [128, KT, D], bf16, kind="ExternalInput")
    asp_ext = nc.dram_tensor("aspect", [128, KT], bf16, kind="ExternalInput")
    b_ext = [nc.dram_tensor("b0", [128, KT], f32, kind="ExternalInput"),
             nc.dram_tensor("b1", [128, KT], f32, kind="ExternalInput")]
    gb_ext = nc.dram_tensor("gb", [128, KT], f32, kind="ExternalInput")
    gam_ext = nc.dram_tensor("gam", [128, L, D], bf16, kind="ExternalInput")
    bet_ext = nc.dram_tensor("bet", [128, L, D], bf16, kind="ExternalInput")
    out_ext = nc.dram_tensor("out", [cfg.p_pad, D], f32, kind="ExternalOutput")

    x1_own = nc.dram_tensor("x1_own", [cfg.p_pad, D], bf16)
    sendbuf = nc.dram_tensor("sendbuf", [cfg.n_cores * bp, D], bf16)
    recvbuf = nc.dram_tensor("recvbuf", [cfg.n_cores * bp, D], bf16)

    with tile.TileContext(nc) as tc:
        with tc.tile_pool(name="single", bufs=1) as single, \
             tc.tile_pool(name="aggT", bufs=1) as aggT_p, \
             tc.tile_pool(name="wrot", bufs=1) as wrot, \
             tc.tile_pool(name="lnc", bufs=1) as lnc, \
             tc.tile_pool(name="msgs", bufs=4) as msgs_p, \
             tc.tile_pool(name="sblk", bufs=4) as s_p, \
             tc.tile_pool(name="aggbf", bufs=3) as aggbf_p, \
             tc.tile_pool(name="colt", bufs=2) as col_p, \
             tc.tile_pool(name="nat", bufs=4) as nat_p, \
             tc.tile_pool(name="lns", bufs=4) as lns_p, \
             tc.tile_pool(name="sgat", bufs=6) as sgat_p, \
             tc.tile_pool(name="psA", bufs=1, space="PSUM") as psA, \
             tc.tile_pool(name="psT", bufs=2, space="PSUM") as psT, \
             tc.tile_pool(name="psM", bufs=3, space="PSUM") as psM:

            ident = single.tile([128, 128], bf16, tag="ident")
            make_identity(nc, ident[:])
            eps_t = single.tile([128, 1], f32, tag="eps")
            nc.vector.memset(eps_t[:], EPS)

            gwt_t = single.tile([128, KT, D], bf16, tag="gwt")
            nc.sync.dma_start(out=gwt_t[:], in_=gwt_ext[:, :, :])
            asp_t = single.tile([128, KT], bf16, tag="asp")
            nc.sync.dma_start(out=asp_t[:], in_=asp_ext[:, :])
            gb_t = single.tile([128, KT], f32, tag="gb")
            nc.sync.dma_start(out=gb_t[:], in_=gb_ext[:, :])
            b_t = single.tile([128, 2, KT], f32, tag="bias")
            nc.sync.dma_start(out=b_t[:, 0, :], in_=b_ext[0][:, :])
            nc.sync.dma_start(out=b_t[:, 1, :], in_=b_ext[1][:, :])
            src_t = single.tile([128, 2, c_pad], i32, tag="src")
            nc.sync.dma_start(out=src_t[:, 1, :], in_=src_ext[1][:, :])
            sidx_t = single.tile([128, nst], i32, tag="sidx")
            nc.sync.dma_start(out=sidx_t[:], in_=sendidx_ext[:, :])
            geff_t = single.tile([128, KT], f32, tag="geff")

            # gate bias fold: geff = aspect @ gate_w[D:] + gate_b
            gwb_t = wrot.tile([128, KT, D], bf16, tag="wl")
            nc.sync.dma_start(out=gwb_t[:], in_=gwb_ext[:, :, :])
            for m in range(KT):
                ps = psM.tile([128, 512], f32, tag="mps")
                for k in range(KT):
                    nc.tensor.matmul(out=ps[:, 0:1],
                                     lhsT=gwb_t[:, k, m * 128:(m + 1) * 128],
                                     rhs=asp_t[:, k:k + 1],
                                     start=(k == 0), stop=(k == KT - 1))
                nc.scalar.activation(out=geff_t[:, m:m + 1], in_=ps[:, 0:1],
                                     func=AF.Identity, bias=gb_t[:, m:m + 1])

            for l in range(L):
                x_src = x0arr_ext if l == 0 else recvbuf
                xold_src = x0own_ext if l == 0 else x1_own

                w_t = wrot.tile([128, KT, D], bf16, tag="wl")
                nc.sync.dma_start(out=w_t[:], in_=w_ext[l][:, :, :])
                gam_t = lnc.tile([128, D], bf16, tag="gam")
                nc.sync.dma_start(out=gam_t[:], in_=gam_ext[:, l, :])
                bet_t = lnc.tile([128, D], bf16, tag="bet")
                nc.sync.dma_start(out=bet_t[:], in_=bet_ext[:, l, :])

                # ---- phase A: gather + scatter + transpose -> aggT
                aggT = aggT_p.tile([128, KT, cfg.p_pad], bf16, tag="aggT")
                mtiles = {}
                for g in range(ngroups):
                    mt = msgs_p.tile([128, R, D], bf16, tag="msgs")
                    if l == 0:
                        nc.sync.dma_start(
                            out=mt[:],
                            in_=x0arr_ext[:, g * R:(g + 1) * R, :])
                    else:
                        for r in range(R):
                            nc.gpsimd.indirect_dma_start(
                                out=mt[:, r, :], out_offset=None,
                                in_=x_src[:, :],
                                in_offset=bass.IndirectOffsetOnAxis(
                                    ap=src_t[:, l, g * R + r:g * R + r + 1], axis=0))
                    mtiles[g] = mt

                for b in range(cfg.nblk):
                    cbb = cb[b]
                    s_t = s_p.tile([128, max(cb) * 128], bf16, tag="sblk")
                    nc.sync.dma_start(
                        out=s_t[:, :cbb * 128],
                        in_=s_ext[:, offs[b] * 128:(offs[b] + cbb) * 128])
                    agg_ps = psA.tile([128, D], f32, tag="aps")
                    for j in range(cbb):
                        c = offs[b] + j
                        mt = mtiles[c // R]
                        jj = c % R
                        nc.tensor.matmul(out=agg_ps[:, 0:512],
                                         lhsT=s_t[:, j * 128:(j + 1) * 128],
                                         rhs=mt[:, jj, 0:512],
                                         start=(j == 0), stop=(j == cbb - 1))
                        nc.tensor.matmul(out=agg_ps[:, 512:D],
                                         lhsT=s_t[:, j * 128:(j + 1) * 128],
                                         rhs=mt[:, jj, 512:D],
                                         start=(j == 0), stop=(j == cbb - 1))
                    agg_bf = aggbf_p.tile([128, D], bf16, tag="aggbf")
                    nc.scalar.copy(out=agg_bf[:], in_=agg_ps[:])
                    for k in range(0, KT, 2):
                        tp = psT.tile([128, 2, 128], bf16, tag="tps")
                        nc.tensor.transpose(out=tp[:, 0, :],
                                            in_=agg_bf[:, k * 128:(k + 1) * 128],
                                            identity=ident[:])
                        nc.tensor.transpose(out=tp[:, 1, :],
                                            in_=agg_bf[:, (k + 1) * 128:(k + 2) * 128],
                                            identity=ident[:])
                        nc.vector.tensor_copy(
                            out=aggT[:, k:k + 2, b * 128:(b + 1) * 128], in_=tp[:])

                # ---- phase B: matmuls + gate + combine + LN per node column
                for (o, w) in cfg.cols:
                    xoldT = col_p.tile([128, KT, 512], bf16, tag="xoldT")
                    for k in range(KT):
                        nc.sync.dma_start_transpose(
                            out=xoldT[:, k, :w],
                            in_=xold_src[o:o + w, k * 128:(k + 1) * 128])
                    xgT = col_p.tile([128, KT, 512], bf16, tag="xgT")
                    for m in range(KT):
                        ps = psM.tile([128, 512], f32, tag="mps")
                        for k in range(KT):
                            nc.tensor.matmul(out=ps[:, :w],
                                             lhsT=w_t[:, k, m * 128:(m + 1) * 128],
                                             rhs=aggT[:, k, o:o + w],
                                             start=(k == 0), stop=(k == KT - 1))
                        nc.scalar.activation(out=xgT[:, m, :w], in_=ps[:, :w],
                                             func=AF.Relu, bias=b_t[:, l, m:m + 1])
                    gT = col_p.tile([128, KT, 512], bf16, tag="gT")
                    for m in range(KT):
                        ps = psM.tile([128, 512], f32, tag="mps")
                        for k in range(KT):
                            nc.tensor.matmul(out=ps[:, :w],
                                             lhsT=gwt_t[:, k, m * 128:(m + 1) * 128],
                                             rhs=xgT[:, k, :w],
                                             start=(k == 0), stop=(k == KT - 1))
                        nc.scalar.activation(out=gT[:, m, :w], in_=ps[:, :w],
                                             func=AF.Sigmoid, bias=geff_t[:, m:m + 1])
                    # combine in place into xgT: xn = g*(xg - xo) + xo
                    nc.vector.tensor_sub(xgT[:, :, :w], xgT[:, :, :w],
                                         xoldT[:, :, :w])
                    nc.vector.tensor_mul(xgT[:, :, :w], gT[:, :, :w],
                                         xgT[:, :, :w])
                    nc.vector.tensor_add(xgT[:, :, :w], xgT[:, :, :w],
                                         xoldT[:, :, :w])
                    # transpose back + LN + store
                    for sub in range(w // 128):
                        nat = nat_p.tile([128, D], bf16, tag="nat")
                        for k in range(0, KT, 2):
                            tp = psT.tile([128, 2, 128], bf16, tag="tps")
                            nc.tensor.transpose(
                                out=tp[:, 0, :],
                                in_=xgT[:, k, sub * 128:(sub + 1) * 128],
                                identity=ident[:])
                            nc.tensor.transpose(
                                out=tp[:, 1, :],
                                in_=xgT[:, k + 1, sub * 128:(sub + 1) * 128],
                                identity=ident[:])
                            nc.vector.tensor_copy(
                                out=nat[:, k * 128:(k + 2) * 128], in_=tp[:])
                        stats = lns_p.tile([128, 3, 6], f32, tag="stats")
                        for gi in range(3):
                            nc.vector.bn_stats(
                                out=stats[:, gi, :],
                                in_=nat[:, 256 * gi:256 * (gi + 1)])
                        mv = lns_p.tile([128, 2], f32, tag="mv")
                        nc.vector.bn_aggr(out=mv[:], in_=stats[:])
                        rstd = lns_p.tile([128, 1], f32, tag="rstd")
                        nc.scalar.activation(out=rstd[:], in_=mv[:, 1:2],
                                             func=AF.Sqrt, bias=eps_t[:])
                        nc.vector.reciprocal(out=rstd[:], in_=rstd[:])
                        nc.vector.tensor_scalar(out=nat[:], in0=nat[:],
                                                scalar1=mv[:, 0:1],
                                                scalar2=rstd[:],
                                                op0=AL.subtract, op1=AL.mult)
                        nc.vector.tensor_mul(nat[:], nat[:], gam_t[:])
                        r0 = o + sub * 128
                        if l == 0:
                            xnb = nat_p.tile([128, D], bf16, tag="natbf")
                            nc.vector.tensor_add(xnb[:], nat[:], bet_t[:])
                            nc.sync.dma_start(out=x1_own[r0:r0 + 128, :],
                                              in_=xnb[:])
                        else:
                            natf = nat_p.tile([128, D], f32, tag="natf")
                            nc.vector.tensor_add(natf[:], nat[:], bet_t[:])
                            nc.sync.dma_start(out=out_ext[r0:r0 + 128, :],
                                              in_=natf[:])

                # ---- between layers: route x1 rows with one AllToAll
                if l == 0:
                    for t in range(nst):
                        st = sgat_p.tile([128, D], bf16, tag="sg")
                        nc.gpsimd.indirect_dma_start(
                            out=st[:], out_offset=None,
                            in_=x1_own[:, :],
                            in_offset=bass.IndirectOffsetOnAxis(
                                ap=sidx_t[:, t:t + 1], axis=0))
                        nc.sync.dma_start(
                            out=sendbuf[t * 128:(t + 1) * 128, :], in_=st[:])
                    nc.gpsimd.collective_compute(
                        "AllToAll",
                        mybir.AluOpType.bypass,
                        replica_groups=[list(range(cfg.n_cores))],
                        ins=[sendbuf[:, :]],
                        outs=[recvbuf[:, :]],
                    )
    nc.compile()
    return nc


# ---------------------------------------------------------------- entry

def _run(inputs, cfg=FULL, trace=False):
    from concourse.bass_utils import run_bass_kernel_spmd
    in_maps, sched = prep(cfg, inputs)
    nc = build(cfg, sched)
    res = run_bass_kernel_spmd(nc, in_maps, core_ids=list(range(cfg.n_cores)),
                               trace=trace)
    outs = [res.results[c]["out"][:cfg.p_local] for c in range(cfg.n_cores)]
    full = np.concatenate(outs, axis=0).astype(np.float32)
    return full, res


def kernel(**inputs):
    out, _ = _run(inputs, FULL, trace=False)
    return out


# revision 17
# speedup vs baseline: 1.2532x; 1.1551x over previous
"""Trainium2 Bass kernel for a 2-layer aspect-gated GCN (AspectOrientedDepGCN).

Strategy (8 NeuronCores, SPMD):
  - Nodes sharded across cores (6250/core, padded to 6272 = 49*128).
  - Edges partitioned by dst partition, sorted by dst, chunked into groups of
    128; scatter-add is computed as one-hot-matrix matmuls accumulated in PSUM.
  - Per layer: gather -> scatter matmuls -> PE transpose to aggT [D, nodes] ->
    weight-stationary matmuls (x_gcn^T, gate^T) -> gated combine in transposed
    layout -> PE transpose back -> LayerNorm (bn_stats) -> DRAM.
  - Layer-2 messages are routed with a single AllToAll of only the rows each
    core actually needs (deduped per (owner, receiver) pair), instead of
    AllGathering the full x1: sender indirect-gathers its x1 rows into
    per-destination send blocks, one 8-way AllToAll moves them, receivers
    indirect-gather messages from the received table.
All matmuls run in bf16 with fp32 PSUM accumulation; LayerNorm in fp32.
"""
import sys

sys.path.insert(0, "/opt/trn_rl_repo")

import numpy as np
import ml_dtypes

D = 768
KT = 6          # D / 128
EPS = 1e-5
L = 2


class Cfg:
    def __init__(self, n_nodes, n_cores, gather_r=4):
        self.n_nodes = n_nodes
        self.n_cores = n_cores
        self.p_local = n_nodes // n_cores          # real nodes per core
        assert self.p_local * n_cores == n_nodes
        self.nblk = (self.p_local + 127) // 128    # 128-node dst blocks
        self.p_pad = self.nblk * 128               # padded nodes per core
        self.n_full = self.p_pad * n_cores
        self.gather_r = gather_r
        # node columns for the transposed main matmuls: 512-wide + remainder
        cols = []
        o = 0
        while o < self.p_pad:
            w = min(512, self.p_pad - o)
            cols.append((o, w))
            o += w
        assert all(w % 128 == 0 for _, w in cols)
        self.cols = cols


FULL = Cfg(50000, 8)


# ---------------------------------------------------------------- host prep

def prep(cfg, inputs):
    """Split edges by dst partition, build chunk schedule + per-core packed
    index / one-hot arrays, AllToAll routing tables, and weight layouts."""
    edge = np.asarray(inputs["edge_index"])
    src_g = edge[0].astype(np.int64)
    dst_g = edge[1].astype(np.int64)
    nc_ = cfg.n_cores

    per_core = []
    counts = np.zeros((nc_, cfg.nblk), np.int64)
    for c in range(nc_):
        m = (dst_g // cfg.p_local) == c
        s = src_g[m]
        d = dst_g[m] - c * cfg.p_local
        order = np.argsort(d, kind="stable")
        s, d = s[order], d[order]
        per_core.append((s, d))
        counts[c] = np.bincount(d // 128, minlength=cfg.nblk)

    # shared chunk schedule: per block, chunks = max over cores
    cb = np.maximum(1, -(-counts.max(axis=0) // 128))   # ceil, min 1
    offs = np.concatenate([[0], np.cumsum(cb)])         # chunk offset per block
    c_total = int(offs[-1])
    ngroups = -(-c_total // cfg.gather_r)
    c_pad = ngroups * cfg.gather_r

    # ---- pack per-core edge slots (slot -> global src id, one-hot S)
    src_packs, s_hosts = [], []
    for c in range(nc_):
        s, d = per_core[c]
        src_slots = np.zeros(c_pad * 128, np.int64)
        scol = np.full(c_pad * 128, -1, np.int64)
        blk = d // 128
        starts = np.concatenate([[0], np.cumsum(np.bincount(blk, minlength=cfg.nblk))])
        for b in range(cfg.nblk):
            e0, e1 = starts[b], starts[b + 1]
            base = offs[b] * 128
            n = e1 - e0
            src_slots[base:base + n] = s[e0:e1]
            scol[base:base + n] = d[e0:e1] - b * 128
        # S one-hot: [128 partitions (edge slot), c_pad*128]
        sh = np.zeros((128, c_pad * 128), ml_dtypes.bfloat16)
        cc = np.arange(c_pad * 128)
        valid = scol >= 0
        p_ = cc[valid] % 128
        ch_ = cc[valid] // 128
        sh[p_, ch_ * 128 + scol[valid]] = 1.0
        # src packed [128, c_pad]; invalid slots hold src 0 (harmless row)
        sp = src_slots.reshape(c_pad, 128).T.copy()
        src_packs.append(sp)
        s_hosts.append(sh)
        mask = valid.reshape(c_pad, 128).T.copy()
        if c == 0:
            masks = [mask]
        else:
            masks.append(mask)

    # ---- AllToAll routing for layer 2
    # pairlist[o][c]: sorted unique global src ids owned by o, needed by c
    pairlist = [[None] * nc_ for _ in range(nc_)]
    bp_need = 0
    for c in range(nc_):
        s = per_core[c][0]
        for o in range(nc_):
            u = np.unique(s[(s // cfg.p_local) == o])
            pairlist[o][c] = u
            bp_need = max(bp_need, len(u))
    bp = -(-bp_need // 16) * 16                   # pair block rows (padded)
    assert (nc_ * bp) % 128 == 0
    nst = nc_ * bp // 128                          # send tiles of 128 rows

    send_idx, recv_idx = [], []
    tile_hi = np.zeros(nst, np.int64)        # max x1 row + 1 per send tile
    for me in range(nc_):
        # sender: sendbuf row (dest*bp + k) <- x1_own row (ascending per pair)
        sidx = np.zeros(nc_ * bp, np.int64)
        for dest in range(nc_):
            u = pairlist[me][dest]
            sidx[dest * bp:dest * bp + len(u)] = u % cfg.p_local
        tile_hi = np.maximum(tile_hi, sidx.reshape(nst, 128).max(axis=1) + 1)
        send_idx.append(np.ascontiguousarray(
            sidx.reshape(nst, 128).T).astype(np.int32))
        # receiver: slot (global src id) -> recvbuf row (owner*bp + rank)
        lut = {}
        for o in range(nc_):
            for k, sgid in enumerate(pairlist[o][me]):
                lut[int(sgid)] = o * bp + k
        sp = src_packs[me]
        ridx = np.zeros_like(sp)
        m = masks[me]
        flat = sp.ravel()
        rflat = np.array([lut.get(int(g), 0) for g in flat], np.int64)
        ridx = rflat.reshape(sp.shape)
        ridx[~m] = 0
        recv_idx.append(ridx.astype(np.int32))

    # round per-tile bound up to a column-group boundary (shared across cores)
    col_hi = np.array([o + w for o, w in cfg.cols])
    tile_hi = col_hi[np.searchsorted(col_hi, tile_hi, side="left")]
    tile_order = np.argsort(tile_hi, kind="stable")

    # ---- weights
    def pack_w(w):  # [D, D] -> [128, KT, D] bf16 (partition-major per k tile)
        return np.ascontiguousarray(
            w.reshape(KT, 128, D).transpose(1, 0, 2)).astype(ml_dtypes.bfloat16)

    def pack_v(v, dt=np.float32):  # [D] -> [128, KT]
        return np.ascontiguousarray(v.reshape(KT, 128).T).astype(dt)

    gcn_w = np.asarray(inputs["gcn_w"], np.float32)
    gate_w = np.asarray(inputs["gate_w"], np.float32)
    x0 = np.asarray(inputs["token_embeddings"], np.float32)

    # padded bf16 replica of x0 and per-core own slices
    x0_rep = np.zeros((cfg.n_full, D), ml_dtypes.bfloat16)
    for c in range(nc_):
        x0_rep[c * cfg.p_pad:c * cfg.p_pad + cfg.p_local] = \
            x0[c * cfg.p_local:(c + 1) * cfg.p_local]

    def _pad_global(g):
        return (g // cfg.p_local) * cfg.p_pad + g % cfg.p_local

    shared = {
        "w0": pack_w(gcn_w[0]),
        "w1": pack_w(gcn_w[1]),
        "gwt": pack_w(gate_w[:D]),
        "gwb": pack_w(gate_w[D:]),
        "aspect": pack_v(np.asarray(inputs["aspect_embedding"]), ml_dtypes.bfloat16),
        "b0": pack_v(np.asarray(inputs["gcn_b"])[0]),
        "b1": pack_v(np.asarray(inputs["gcn_b"])[1]),
        "gb": pack_v(np.asarray(inputs["gate_b"])),
        "gam": np.broadcast_to(
            np.asarray(inputs["ln_gamma"]).astype(ml_dtypes.bfloat16)[None],
            (128, L, D)).copy(),
        "bet": np.broadcast_to(
            np.asarray(inputs["ln_beta"]).astype(ml_dtypes.bfloat16)[None],
            (128, L, D)).copy(),
    }
    in_maps = []
    for c in range(nc_):
        m = dict(shared)
        m["x0own"] = np.ascontiguousarray(
            x0_rep[c * cfg.p_pad:(c + 1) * cfg.p_pad])
        src1 = _pad_global(src_packs[c]).astype(np.int32)
        # L1 messages pre-arranged, partition-major: contiguous per-partition
        m["x0arr"] = np.ascontiguousarray(x0_rep[src1])   # [128, c_pad, D]
        m["src2"] = recv_idx[c]
        m["sendidx"] = send_idx[c]
        m["smat"] = s_hosts[c]
        in_maps.append(m)

    sched = {"cb": cb.astype(int).tolist(), "offs": offs.astype(int).tolist(),
             "c_total": c_total, "c_pad": c_pad, "ngroups": ngroups,
             "bp": bp, "nst": nst,
             "tile_hi": tile_hi.astype(int).tolist(),
             "tile_order": tile_order.astype(int).tolist()}
    return in_maps, sched


# ---------------------------------------------------------------- builder

def build(cfg, sched, dbg=False):
    import concourse.bass as bass
    import concourse.tile as tile
    from concourse import bacc, mybir
    from concourse.masks import make_identity

    f32 = mybir.dt.float32
    bf16 = mybir.dt.bfloat16
    i32 = mybir.dt.int32
    AF = mybir.ActivationFunctionType
    AL = mybir.AluOpType

    cb, offs = sched["cb"], sched["offs"]
    c_pad, ngroups = sched["c_pad"], sched["ngroups"]
    bp, nst = sched["bp"], sched["nst"]
    R = cfg.gather_r

    nc = bacc.Bacc("TRN2", target_bir_lowering=False, debug=False,
                   num_devices=cfg.n_cores)

    x0arr_ext = nc.dram_tensor("x0arr", [128, c_pad, D], bf16,
                               kind="ExternalInput")
    x0own_ext = nc.dram_tensor("x0own", [cfg.p_pad, D], bf16, kind="ExternalInput")
    src_ext = [None,
               nc.dram_tensor("src2", [128, c_pad], i32, kind="ExternalInput")]
    sendidx_ext = nc.dram_tensor("sendidx", [128, nst], i32, kind="ExternalInput")
    s_ext = nc.dram_tensor("smat", [128, c_pad * 128], bf16, kind="ExternalInput")
    w_ext = [nc.dram_tensor("w0", [128, KT, D], bf16, kind="ExternalInput"),
             nc.dram_tensor("w1", [128, KT, D], bf16, kind="ExternalInput")]
    gwt_ext = nc.dram_tensor("gwt", [128, KT, D], bf16, kind="ExternalInput")
    gwb_ext = nc.dram_tensor("gwb", [128, KT, D], bf16, kind="ExternalInput")
    asp_ext = nc.dram_tensor("aspect", [128, KT], bf16, kind="ExternalInput")
    b_ext = [nc.dram_tensor("b0", [128, KT], f32, kind="ExternalInput"),
             nc.dram_tensor("b1", [128, KT], f32, kind="ExternalInput")]
    gb_ext = nc.dram_tensor("gb", [128, KT], f32, kind="ExternalInput")
    gam_ext = nc.dram_tensor("gam", [128, L, D], bf16, kind="ExternalInput")
    bet_ext = nc.dram_tensor("bet", [128, L, D], bf16, kind="ExternalInput")
    out_ext = nc.dram_tensor("out", [cfg.p_pad, D], f32, kind="ExternalOutput")

    x1_own = nc.dram_tensor("x1_own", [cfg.p_pad, D], bf16)
    sendbuf = nc.dram_tensor("sendbuf", [cfg.n_cores * bp, D], bf16)
    recvbuf = nc.dram_tensor("recvbuf", [cfg.n_cores * bp, D], bf16)

    with tile.TileContext(nc) as tc:
        with tc.tile_pool(name="single", bufs=1) as single, \
             tc.tile_pool(name="aggT", bufs=1) as aggT_p, \
             tc.tile_pool(name="wrot", bufs=1) as wrot, \
             tc.tile_pool(name="lnc", bufs=1) as lnc, \
             tc.tile_pool(name="msgs", bufs=4) as msgs_p, \
             tc.tile_pool(name="sblk", bufs=4) as s_p, \
             tc.tile_pool(name="colt", bufs=2) as col_p, \
             tc.tile_pool(name="nat", bufs=2) as nat_p, \
             tc.tile_pool(name="lns", bufs=4) as lns_p, \
             tc.tile_pool(name="sgat", bufs=6) as sgat_p, \
             tc.tile_pool(name="psA", bufs=2, space="PSUM") as psA, \
             tc.tile_pool(name="psT", bufs=2, space="PSUM") as psT, \
             tc.tile_pool(name="psM", bufs=2, space="PSUM") as psM:

            ident = single.tile([128, 128], bf16, tag="ident")
            make_identity(nc, ident[:])

            gwt_t = single.tile([128, KT, D], bf16, tag="gwt")
            nc.sync.dma_start(out=gwt_t[:], in_=gwt_ext[:, :, :])
            asp_t = single.tile([128, KT], bf16, tag="asp")
            nc.sync.dma_start(out=asp_t[:], in_=asp_ext[:, :])
            gb_t = single.tile([128, KT], f32, tag="gb")
            nc.sync.dma_start(out=gb_t[:], in_=gb_ext[:, :])
            b_t = single.tile([128, 2, KT], f32, tag="bias")
            nc.sync.dma_start(out=b_t[:, 0, :], in_=b_ext[0][:, :])
            nc.sync.dma_start(out=b_t[:, 1, :], in_=b_ext[1][:, :])
            src_t = single.tile([128, 2, c_pad], i32, tag="src")
            nc.sync.dma_start(out=src_t[:, 1, :], in_=src_ext[1][:, :])
            sidx_t = single.tile([128, nst], i32, tag="sidx")
            nc.sync.dma_start(out=sidx_t[:], in_=sendidx_ext[:, :])
            geff_t = single.tile([128, KT], f32, tag="geff")

            # gate bias fold: geff = aspect @ gate_w[D:] + gate_b
            gwb_t = wrot.tile([128, KT, D], bf16, tag="wl")
            nc.sync.dma_start(out=gwb_t[:], in_=gwb_ext[:, :, :])
            for m in range(KT):
                ps = psM.tile([128, 512], f32, tag="mps")
                for k in range(KT):
                    nc.tensor.matmul(out=ps[:, 0:1],
                                     lhsT=gwb_t[:, k, m * 128:(m + 1) * 128],
                                     rhs=asp_t[:, k:k + 1],
                                     start=(k == 0), stop=(k == KT - 1))
                nc.scalar.activation(out=geff_t[:, m:m + 1], in_=ps[:, 0:1],
                                     func=AF.Identity, bias=gb_t[:, m:m + 1])

            for l in range(L):
                x_src = x0arr_ext if l == 0 else recvbuf
                xold_src = x0own_ext if l == 0 else x1_own

                w_t = wrot.tile([128, KT, D], bf16, tag="wl")
                nc.sync.dma_start(out=w_t[:], in_=w_ext[l][:, :, :])
                gam_t = lnc.tile([128, D], bf16, tag="gam")
                nc.sync.dma_start(out=gam_t[:], in_=gam_ext[:, l, :])
                bet_t = lnc.tile([128, D], bf16, tag="bet")
                nc.sync.dma_start(out=bet_t[:], in_=bet_ext[:, l, :])

                # ---- phase A: gather + scatter + transpose -> aggT
                aggT = aggT_p.tile([128, KT, cfg.p_pad], bf16, tag="aggT")
                mtiles = {}
                for g in range(ngroups):
                    mt = msgs_p.tile([128, R, D], bf16, tag="msgs")
                    if l == 0:
                        nc.sync.dma_start(
                            out=mt[:],
                            in_=x0arr_ext[:, g * R:(g + 1) * R, :])
                    else:
                        for r in range(R):
                            nc.gpsimd.indirect_dma_start(
                                out=mt[:, r, :], out_offset=None,
                                in_=x_src[:, :],
                                in_offset=bass.IndirectOffsetOnAxis(
                                    ap=src_t[:, l, g * R + r:g * R + r + 1], axis=0))
                    mtiles[g] = mt

                for b in range(cfg.nblk):
                    cbb = cb[b]
                    s_t = s_p.tile([128, max(cb) * 128], bf16, tag="sblk")
                    # L2 S loads go on the scalar queue so SP can prefetch
                    # xoldT tiles during the AllToAll window
                    s_eng = nc.sync if l == 0 else nc.scalar
                    s_eng.dma_start(
                        out=s_t[:, :cbb * 128],
                        in_=s_ext[:, offs[b] * 128:(offs[b] + cbb) * 128])
                    # scatter matmuls emit aggT directly:
                    # out[feat, dst] = sum_slots msgs[slot, feat] * S[slot, dst]
                    atp = psA.tile([128, KT, 128], f32, tag="aps")
                    for k in range(KT):
                        for j in range(cbb):
                            c = offs[b] + j
                            mt = mtiles[c // R]
                            jj = c % R
                            nc.tensor.matmul(
                                out=atp[:, k, :],
                                lhsT=mt[:, jj, k * 128:(k + 1) * 128],
                                rhs=s_t[:, j * 128:(j + 1) * 128],
                                start=(j == 0), stop=(j == cbb - 1))
                    nc.scalar.copy(
                        out=aggT[:, :, b * 128:(b + 1) * 128], in_=atp[:])

                # ---- phase B: matmuls + gate + combine + LN per node column
                for (o, w) in cfg.cols:
                    xoldT = col_p.tile([128, KT, 512], bf16, tag="xoldT")
                    for k in range(KT):
                        nc.sync.dma_start_transpose(
                            out=xoldT[:, k, :w],
                            in_=xold_src[o:o + w, k * 128:(k + 1) * 128])
                    xgT = col_p.tile([128, KT, 512], bf16, tag="xgT")
                    for m in range(KT):
                        ps = psM.tile([128, 512], f32, tag="mps")
                        for k in range(KT):
                            nc.tensor.matmul(out=ps[:, :w],
                                             lhsT=w_t[:, k, m * 128:(m + 1) * 128],
                                             rhs=aggT[:, k, o:o + w],
                                             start=(k == 0), stop=(k == KT - 1))
                        nc.scalar.activation(out=xgT[:, m, :w], in_=ps[:, :w],
                                             func=AF.Relu, bias=b_t[:, l, m:m + 1])
                    gT = col_p.tile([128, KT, 512], bf16, tag="gT")
                    for m in range(KT):
                        ps = psM.tile([128, 512], f32, tag="mps")
                        for k in range(KT):
                            nc.tensor.matmul(out=ps[:, :w],
                                             lhsT=gwt_t[:, k, m * 128:(m + 1) * 128],
                                             rhs=xgT[:, k, :w],
                                             start=(k == 0), stop=(k == KT - 1))
                        nc.scalar.activation(out=gT[:, m, :w], in_=ps[:, :w],
                                             func=AF.Sigmoid, bias=geff_t[:, m:m + 1])
                    # combine in place into xgT: xn = g*(xg - xo) + xo
                    nc.gpsimd.tensor_sub(xgT[:, :, :w], xgT[:, :, :w],
                                         xoldT[:, :, :w])
                    nc.vector.tensor_mul(xgT[:, :, :w], gT[:, :, :w],
                                         xgT[:, :, :w])
                    nc.gpsimd.tensor_add(xgT[:, :, :w], xgT[:, :, :w],
                                         xoldT[:, :, :w])
                    # transpose back + LN stats per 128-node sub-block
                    nsub = w // 128
                    natc = nat_p.tile([128, 4, D], bf16, tag="nat")
                    mvc = lns_p.tile([128, 4, 2], f32, tag="mv")
                    for sub in range(nsub):
                        tp = psT.tile([128, KT, 128], bf16, tag="tps")
                        for k in range(KT):
                            nc.tensor.transpose(
                                out=tp[:, k, :],
                                in_=xgT[:, k, sub * 128:(sub + 1) * 128],
                                identity=ident[:])
                        nc.vector.tensor_copy(out=natc[:, sub, :], in_=tp[:])
                        stats = lns_p.tile([128, 2, 6], f32, tag="stats")
                        nc.vector.bn_stats(out=stats[:, 0, :],
                                           in_=natc[:, sub, 0:512])
                        nc.vector.bn_stats(out=stats[:, 1, :],
                                           in_=natc[:, sub, 512:768])
                        nc.vector.bn_aggr(out=mvc[:, sub, :], in_=stats[:])
                    # batched rstd = rsqrt(var + eps) on DVE (no act tables)
                    vr = lns_p.tile([128, 4], f32, tag="vr")
                    ys = lns_p.tile([128, 4], f32, tag="ys")
                    tmp = lns_p.tile([128, 4], f32, tag="tmp")
                    nc.vector.tensor_scalar(out=vr[:, :nsub],
                                            in0=mvc[:, :nsub, 1],
                                            scalar1=EPS, scalar2=None,
                                            op0=AL.add)
                    yi = ys[:].bitcast(i32)
                    nc.vector.tensor_scalar(out=yi[:, :nsub],
                                            in0=vr[:, :nsub].bitcast(i32),
                                            scalar1=1, scalar2=None,
                                            op0=AL.logical_shift_right)
                    nc.vector.tensor_scalar(out=yi[:, :nsub],
                                            in0=yi[:, :nsub],
                                            scalar1=-1, scalar2=0x5f3759df,
                                            op0=AL.mult, op1=AL.add)
                    for _ in range(2):
                        nc.vector.tensor_mul(tmp[:, :nsub], ys[:, :nsub],
                                             ys[:, :nsub])
                        nc.vector.tensor_mul(tmp[:, :nsub], tmp[:, :nsub],
                                             vr[:, :nsub])
                        nc.vector.tensor_scalar(out=tmp[:, :nsub],
                                                in0=tmp[:, :nsub],
                                                scalar1=-0.5, scalar2=1.5,
                                                op0=AL.mult, op1=AL.add)
                        nc.vector.tensor_mul(ys[:, :nsub], ys[:, :nsub],
                                             tmp[:, :nsub])
                    # apply LN + store
                    for sub in range(nsub):
                        nc.vector.tensor_scalar(out=natc[:, sub, :],
                                                in0=natc[:, sub, :],
                                                scalar1=mvc[:, sub, 0:1],
                                                scalar2=ys[:, sub:sub + 1],
                                                op0=AL.subtract, op1=AL.mult)
                        nc.vector.tensor_mul(natc[:, sub, :], natc[:, sub, :],
                                             gam_t[:])
                        r0 = o + sub * 128
                        if l == 0:
                            xnb = nat_p.tile([128, D], bf16, tag="natbf")
                            nc.gpsimd.tensor_add(xnb[:], natc[:, sub, :],
                                                 bet_t[:])
                            nc.sync.dma_start(out=x1_own[r0:r0 + 128, :],
                                              in_=xnb[:])
                        else:
                            natf = nat_p.tile([128, D], f32, tag="natf")
                            nc.gpsimd.tensor_add(natf[:], natc[:, sub, :],
                                                 bet_t[:])
                            nc.sync.dma_start(out=out_ext[r0:r0 + 128, :],
                                              in_=natf[:])

                # ---- between layers: route x1 rows with one AllToAll.
                # Tiles are issued in ascending x1-row-bound order with a
                # bounded source AP, so gathers start as soon as the column
                # groups covering their rows are written (overlap phase B).
                if l == 0:
                    for t in sched["tile_order"]:
                        hi = sched["tile_hi"][t]
                        st = sgat_p.tile([128, D], bf16, tag="sg")
                        nc.gpsimd.indirect_dma_start(
                            out=st[:], out_offset=None,
                            in_=x1_own[0:hi, :],
                            in_offset=bass.IndirectOffsetOnAxis(
                                ap=sidx_t[:, t:t + 1], axis=0))
                        nc.sync.dma_start(
                            out=sendbuf[t * 128:(t + 1) * 128, :], in_=st[:])
                    nc.gpsimd.collective_compute(
                        "AllToAll",
                        mybir.AluOpType.bypass,
                        replica_groups=[list(range(cfg.n_cores))],
                        ins=[sendbuf[:, :]],
                        outs=[recvbuf[:, :]],
                    )
    nc.compile()
    return nc


# ---------------------------------------------------------------- entry

def _run(inputs, cfg=FULL, trace=False):
    from concourse.bass_utils import run_bass_kernel_spmd
    in_maps, sched = prep(cfg, inputs)
    nc = build(cfg, sched)
    res = run_bass_kernel_spmd(nc, in_maps, core_ids=list(range(cfg.n_cores)),
                               trace=trace)
    outs = [res.results[c]["out"][:cfg.p_local] for c in range(cfg.n_cores)]
    full = np.concatenate(outs, axis=0).astype(np.float32)
    return full, res


def kernel(**inputs):
    out, _ = _run(inputs, FULL, trace=False)
    return out
